# revision 15
# baseline (speedup 1.0000x reference)
"""Conv2d 3x3 (pad 1, stride 1) + bias on 8 Trainium2 cores.

Problem: x [32,128,56,56] f32, weights [256,128,3,3] f32, bias [256] f32
         -> out [32,256,56,56] f32.

Strategy
--------
Data-parallel over batch (4 images/core) + 1D Winograd F(2,3) along W.

For each output pair (2u, 2u+1) and each vertical tap kh, the 3-tap
horizontal conv costs 4 multiplies instead of 6: with d0..d3 the 4
padded inputs around the pair,
  t0 = d0-d2, t1 = d1+d2, t2 = d2-d1, t3 = d1-d3
  m_p = sum_cin sum_kh w'_p[kh] * t_p[row r+kh]
        (w'_0=g0, w'_1=(g0+g1+g2)/2, w'_2=(g0-g1+g2)/2, w'_3=g2)
  out[2u]   = m0+m1+m2+bias
  out[2u+1] = m1-m2-m3+bias
PE work drops from 9 to 6 matmul-columns per output pixel (and the
junk 57-stride column of a direct kernel disappears): 150.5K cols/core
= 62.7us at 2.4GHz vs 95.8us direct.

Layout: the host splits padded rows into even/odd column planes stored
row-major ([58 rows][2 planes][29 cols]) so (a) the four t-plane
transforms are contiguous packed-bf16 tensor_tensor ops on DVE (2x
mode) and (b) row-chunk DMAs write contiguous ranges (the tile dep
tracker uses bounding intervals; an interleaved layout creates false
chunk->transform deps). The t-planes [cin, 58*28] use flat row-stride
28: vertical tap kh of a group at flat col lo is the constant offset
lo + kh*28, so 392-col matmuls run seamlessly across row boundaries.

Per group of N=392 pair-cols: 12 matmuls (4 m-banks x 3 kh) accumulate
in 4 PSUM banks; 2 groups double-buffer across the 8 banks. GpSimd
cannot read PSUM and two-tensor DVE ops may read at most one PSUM
operand, so the A^T combine is:
  ACT:    a1 = Ident(m1+bias), a2 = Copy(m2), a3 = Copy(-m3)
  DVE:    w1 = a1-a2 (sbuf 2x), u0 = m0+a1 (1 psum), out0 = u0+a2
  GpSimd: out1 = w1+a3 (sbuf only)
Outputs stay as separate even/odd bf16 planes, one DMA per plane per
group (per-plane DRAM ranges are contiguous -> no false WAW chain
between group output DMAs); the host interleaves and widens to f32
(tolerance is 2e-2; bf16 out lands ~7e-3).

Startup: warmup matmuls on an uninitialized (dep-free) tile ramp the
PE clock from the first instruction; the first DMA wave (chunk 0 on
SP; half-0 weights + chunk 1 + bias on ACT) lands while they run.
Transfers not needed before ~10us (chunks 2-4, half-1 weights, image
prefetches) are gated behind warmup/first-drain WAW touches so the
Tile scheduler cannot hoist them into the critical wave. Transforms
for image b+1 run on DVE as half-planes spread over 6 group slots of
image b; the split image prefetch feeds them. The final half tapers
(392,392,392,280,112) and its last drain chain avoids the (possibly
backlogged) GpSimd queue and ships its two planes on separate queues.
"""

import numpy as np
import ml_dtypes

import concourse.bacc as bacc
import concourse.mybir as mybir
import concourse.tile as tile
from concourse.bass_utils import run_bass_kernel_spmd
from concourse.alu_op_type import AluOpType

B, CIN, H, W = 32, 128, 56, 56
COUT = 256
NCORES = 8
BLOC = B // NCORES  # images per core
NR = H + 2  # 58 padded rows
PW = W // 2 + 1  # 29 even/odd plane cols
PC = W // 2  # 28 output pairs per row
NPAIR = H * PC  # 1568 output pair-cols per image-half
NWARM = 4

# Weight stationary order per half = first-use order: m1, m2, m0, m3.
WORDER = [1, 2, 0, 3]
PIDX = {p: i for i, p in enumerate(WORDER)}

NORM_GROUPS = [(0, 392), (392, 392), (784, 392), (1176, 392)]
# Image 0 half 0: start-taper so the first matmuls need only 7 input
# rows (the first DMA chunk).
FIRST_GROUPS = [(0, 112), (112, 280), (392, 392), (784, 392), (1176, 392)]
# Last half: end-taper so the final drain + output DMA chain after the
# last matmul is as short as possible.
LAST_GROUPS = [(0, 392), (392, 392), (784, 392), (1176, 280), (1456, 112)]
# Image 0 xe/xo row chunks (DMA + transform granularity).
CHUNKS0 = [(0, 7), (7, 17), (17, 31), (31, 45), (45, 58)]

# Transform schedule for image b+1 during image b, as
# (h, gi) -> [(plane, row_lo, row_hi), ...]. Plane 1 first (used first
# by the next image's group 0), planes 0/3 last (their first use is
# deepest into the next image's group 0).
TSCHED = {
    (0, 2): [(1, 0, 29)],
    (0, 3): [(1, 29, 58)],
    (1, 0): [(2, 0, 29)],
    (1, 1): [(2, 29, 58)],
    (1, 2): [(0, 0, 29), (0, 29, 58)],
    (1, 3): [(3, 0, 29), (3, 29, 58)],
}

_nc_cache = None


def _build():
    f32 = mybir.dt.float32
    bf16 = mybir.dt.bfloat16
    COPY = mybir.ActivationFunctionType.Copy
    IDENT = mybir.ActivationFunctionType.Identity
    nc = bacc.Bacc("TRN2", target_bir_lowering=False)
    x_d = nc.dram_tensor("xeo", [BLOC, CIN, NR, 2, PW], bf16, kind="ExternalInput")
    w_d = nc.dram_tensor("wT", [CIN, 2 * 4 * 3 * 128], bf16, kind="ExternalInput")
    b_d = nc.dram_tensor("bias2", [128, 2], f32, kind="ExternalInput")
    o_d = nc.dram_tensor("out", [BLOC, 2, 128, 2, NPAIR], bf16, kind="ExternalOutput")

    def wcol(h, p, kh):
        return ((h * 4 + PIDX[p]) * 3 + kh) * 128

    with tile.TileContext(nc) as tc:
        with (
            tc.tile_pool(name="wpool", bufs=1) as wpool,
            tc.tile_pool(name="xpool", bufs=2) as xpool,
            tc.tile_pool(name="tpool", bufs=8) as tpool,
            tc.tile_pool(name="upool", bufs=3) as upool,
            tc.tile_pool(name="vpool", bufs=3) as vpool,
            tc.tile_pool(name="opool", bufs=4) as opool,
            tc.tile_pool(name="psum", bufs=8, space="PSUM") as psum,
        ):
            wsb = wpool.tile([CIN, 2 * 4 * 3 * 128], bf16)
            bsb = wpool.tile([128, 2], f32)
            wub = wpool.tile([128, 512], bf16)
            dmy = wpool.tile([128, 2], bf16)
            nc.vector.memset(wub[:], 0.0)
            # Dummy Identity activation: pulls the ~1.3us activation
            # table load to the front of the ACT queue (its engine
            # queue depth is 0, so a late table load would stall it).
            nc.scalar.activation(dmy[:], wub[:, :2], IDENT)

            xeos = [xpool.tile([CIN, NR, 2, PW], bf16, tag="xeo", name="xeo0")]
            tpls = [
                [
                    tpool.tile([CIN, NR, PC], bf16, tag="tp", name=f"tp0_{p}")
                    for p in range(4)
                ]
            ]

            # PE warmup on the uninitialized tile: matmul 1 issues at
            # the first possible cycle and its completion ungates the
            # non-critical DMAs below; 2-4 keep the clock ramping while
            # the first chunks land and transform.
            wup = psum.tile([128, 512], f32, tag="pt", name="wup")
            nc.tensor.matmul(
                wup[:], lhsT=wub[:, :128], rhs=wub[:], start=True, stop=True
            )
            # WAW touches: gate chunks 2-4 + half-1 weights behind
            # warmup matmul 1 (the Tile scheduler hoists dep-free DMAs
            # past queue order, so position alone cannot hold them out
            # of the critical first wave).
            nc.vector.tensor_scalar_mul(wsb[:, 1536:1538], wup[:, :2], 0.0)
            xeo0 = xeos[0]
            for (r0, r1) in CHUNKS0[2:]:
                nc.vector.tensor_scalar_mul(
                    xeo0[:, r0, 0, 0:2], wup[:, :2], 0.0
                )
            for _ in range(NWARM - 1):
                nc.tensor.matmul(
                    wup[:], lhsT=wub[:, :128], rhs=wub[:], start=True, stop=True
                )

            # Startup DMA wave. SP: chunk 0, then gated chunks 2/4.
            # ACT: half-0 weights, chunk 1, bias, chunk 3, gated half-1
            # weights. Two queues generate descriptors in parallel.
            c0, c1 = CHUNKS0[0], CHUNKS0[1]
            c2, c3, c4 = CHUNKS0[2], CHUNKS0[3], CHUNKS0[4]
            nc.sync.dma_start(
                xeo0[:, c0[0] : c0[1]], x_d[0, :, c0[0] : c0[1]]
            )
            nc.scalar.dma_start(wsb[:, 0:1536], w_d[:, 0:1536])
            nc.sync.dma_start(xeo0[:, c2[0] : c2[1]], x_d[0, :, c2[0] : c2[1]])
            nc.scalar.dma_start(
                xeo0[:, c1[0] : c1[1]], x_d[0, :, c1[0] : c1[1]]
            )
            nc.sync.dma_start(xeo0[:, c4[0] : c4[1]], x_d[0, :, c4[0] : c4[1]])
            nc.scalar.dma_start(bsb[:], b_d[:])
            nc.scalar.dma_start(
                xeo0[:, c3[0] : c3[1]], x_d[0, :, c3[0] : c3[1]]
            )
            nc.scalar.dma_start(wsb[:, 1536:], w_d[:, 1536:])

            def transform(bi, r0, r1, only=None):
                """t-plane rows [r0,r1) for image slot bi (DVE)."""
                xeo = xeos[bi]
                tp = tpls[bi]
                xe = lambda a, b_: xeo[:, r0:r1, 0, a:b_]
                xo = lambda a, b_: xeo[:, r0:r1, 1, a:b_]
                ops = {
                    0: (nc.vector.tensor_sub, xe(0, PC), xe(1, PC + 1)),
                    1: (nc.vector.tensor_add, xo(0, PC), xe(1, PC + 1)),
                    2: (nc.vector.tensor_sub, xe(1, PC + 1), xo(0, PC)),
                    3: (nc.vector.tensor_sub, xo(0, PC), xo(1, PC + 1)),
                }
                order = [only] if only is not None else WORDER
                for p in order:
                    fn, a, b_ = ops[p]
                    fn(tp[p][:, r0:r1, :], a, b_)

            transform(0, *CHUNKS0[0])
            transform(0, *CHUNKS0[1])

            def do_group(b, h, lo, n, last_group=False, after_drains=None,
                         prefetch=False):
                tp = tpls[b]
                flat = [tp[p][:].rearrange("c r u -> c (r u)") for p in range(4)]
                pts = {}
                for p in WORDER:
                    pts[p] = psum.tile(
                        [128, 392], f32, tag="pt", name=f"pt_b{b}h{h}l{lo}p{p}"
                    )
                    for kh in range(3):
                        c = wcol(h, p, kh)
                        nc.tensor.matmul(
                            pts[p][:, :n],
                            lhsT=wsb[:, c : c + 128],
                            rhs=flat[p][:, lo + kh * PC : lo + kh * PC + n],
                            start=(kh == 0),
                            stop=(kh == 2),
                        )
                a1 = vpool.tile([128, 392], bf16, tag="a1")
                a2 = vpool.tile([128, 392], bf16, tag="a2")
                a3 = vpool.tile([128, 392], bf16, tag="a3")
                u0 = upool.tile([128, 392], bf16, tag="u0")
                w1 = upool.tile([128, 392], bf16, tag="w1")
                ot = opool.tile([128, 2, 392], bf16, tag="ot")
                bvec = bsb[:, h : h + 1]
                nc.scalar.activation(a1[:, :n], pts[1][:, :n], IDENT, bias=bvec)
                nc.scalar.activation(a2[:, :n], pts[2][:, :n], COPY)
                nc.scalar.activation(a3[:, :n], pts[3][:, :n], COPY, scale=-1.0)
                # out0 = (m0 + a1) + a2 ; out1 = (a1 - a2) + a3
                nc.vector.tensor_sub(w1[:, :n], a1[:, :n], a2[:, :n])
                nc.vector.tensor_add(u0[:, :n], pts[0][:, :n], a1[:, :n])
                nc.vector.tensor_add(ot[:, 0, :n], u0[:, :n], a2[:, :n])
                out1_eng = nc.vector if last_group else nc.gpsimd
                out1_eng.tensor_add(ot[:, 1, :n], w1[:, :n], a3[:, :n])
                oq1 = nc.scalar if last_group else nc.sync
                nc.sync.dma_start(o_d[b, h, :, 0, lo : lo + n], ot[:, 0, :n])
                oq1.dma_start(o_d[b, h, :, 1, lo : lo + n], ot[:, 1, :n])
                if prefetch:
                    # Prefetch next image's xe/xo in two halves, each
                    # gated behind this group's first output plane (an
                    # early 861KB prefetch would starve the transfers
                    # gating the PE; the halves let the first
                    # transforms start 1.2us sooner).
                    xeon = xpool.tile(
                        [CIN, NR, 2, PW], bf16, tag="xeo", name=f"xeo{b+1}"
                    )
                    xeos.append(xeon)
                    tpls.append(
                        [
                            tpool.tile(
                                [CIN, NR, PC], bf16, tag="tp", name=f"tp{b+1}_{p}"
                            )
                            for p in range(4)
                        ]
                    )
                    nc.gpsimd.tensor_scalar_mul(
                        xeon[:, 0, 0, 0:2], ot[:, 0, 0:2], 0.0
                    )
                    nc.gpsimd.tensor_scalar_mul(
                        xeon[:, 29, 0, 0:2], ot[:, 0, 2:4], 0.0
                    )
                    nc.sync.dma_start(xeon[:, 0:29], x_d[b + 1, :, 0:29])
                    nc.sync.dma_start(xeon[:, 29:58], x_d[b + 1, :, 29:58])
                if after_drains is not None:
                    after_drains()

            for b in range(BLOC):
                for h in range(2):
                    if b == 0 and h == 0:
                        groups = FIRST_GROUPS
                    elif b == BLOC - 1 and h == 1:
                        groups = LAST_GROUPS
                    else:
                        groups = NORM_GROUPS
                    for gi, (lo, n) in enumerate(groups):
                        after = None
                        if b == 0 and h == 0 and gi < 3:
                            r0, r1 = CHUNKS0[gi + 2]
                            after = lambda r0=r0, r1=r1: transform(0, r0, r1)
                        elif b + 1 < BLOC:
                            # Image 0 half 0 is tapered into 5 groups;
                            # its last two slots take the (0,2)/(0,3)
                            # transform jobs.
                            gkey = (h, gi - 1 if (b == 0 and h == 0) else gi)
                            jobs = TSCHED.get(gkey)
                            if jobs:
                                after = lambda b=b, jobs=jobs: [
                                    transform(b + 1, r0, r1, only=p)
                                    for (p, r0, r1) in jobs
                                ]
                        do_group(
                            b, h, lo, n,
                            last_group=(
                                b == BLOC - 1 and h == 1 and gi == len(groups) - 1
                            ),
                            after_drains=after,
                            prefetch=(h == 0 and gi == 0 and b + 1 < BLOC),
                        )

    nc.compile()
    return nc


def _get_nc():
    global _nc_cache
    if _nc_cache is None:
        _nc_cache = _build()
    return _nc_cache


def _prep_inputs(x, weights, bias):
    x = np.asarray(x, dtype=np.float32)
    weights = np.asarray(weights, dtype=np.float32)
    bias = np.ascontiguousarray(np.asarray(bias, dtype=np.float32))

    xb = x.astype(ml_dtypes.bfloat16)
    xpad = np.pad(xb, ((0, 0), (0, 0), (1, 1), (1, 1)))  # [B,C,58,58]
    xe = xpad[:, :, :, 0::2]  # [B,C,58,29]
    xo = xpad[:, :, :, 1::2]
    xeo = np.ascontiguousarray(np.stack([xe, xo], axis=3))  # [B,C,58,2,29]

    g = weights.reshape(2, 128, CIN, 3, 3)  # [h, co, cin, kh, kw]
    w0 = g[..., 0]
    w1 = (g[..., 0] + g[..., 1] + g[..., 2]) * 0.5
    w2 = (g[..., 0] - g[..., 1] + g[..., 2]) * 0.5
    w3 = g[..., 2]
    wlist = [w0, w1, w2, w3]
    # stack in WORDER; axes [h, p, co, cin, kh] -> [cin, h, p, kh, co]
    wp = np.stack([wlist[p] for p in WORDER], axis=1)
    wT = np.ascontiguousarray(wp.transpose(3, 0, 1, 4, 2)).reshape(
        CIN, 2 * 4 * 3 * 128
    ).astype(ml_dtypes.bfloat16)
    b2 = np.ascontiguousarray(bias.reshape(2, 128).T)  # b2[p,h] = bias[h*128+p]

    return [
        {
            "xeo": np.ascontiguousarray(xeo[i * BLOC : (i + 1) * BLOC]),
            "wT": wT,
            "bias2": b2,
        }
        for i in range(NCORES)
    ]


def _run(inputs, trace=False):
    in_maps = _prep_inputs(inputs["x"], inputs["weights"], inputs["bias"])
    res = run_bass_kernel_spmd(
        _get_nc(), in_maps, core_ids=list(range(NCORES)), trace=trace
    )
    o = np.concatenate([np.asarray(r["out"]) for r in res.results], axis=0)
    # [B, 2h, 128co, 2pl, 1568] bf16 -> [B, 256, 56, 56] f32
    o = o.astype(np.float32).reshape(B, 2, 128, 2, H, PC)
    o = o.transpose(0, 1, 2, 4, 5, 3).reshape(B, COUT, H, W)
    return np.ascontiguousarray(o), res


def kernel(x, weights, bias):
    out, _ = _run({"x": x, "weights": weights, "bias": bias})
    return out


# revision 18
# speedup vs baseline: 1.1772x; 1.1772x over previous
"""Conv2d 3x3 (pad 1, stride 1) + bias on 8 Trainium2 cores.

Problem: x [32,128,56,56] f32, weights [256,128,3,3] f32, bias [256] f32
         -> out [32,256,56,56] f32.

Strategy
--------
Data-parallel over batch (4 images/core) + 1D Winograd F(2,3) along W.

For each output pair (2u, 2u+1) and each vertical tap kh, the 3-tap
horizontal conv costs 4 multiplies instead of 6: with d0..d3 the 4
padded inputs around the pair,
  t0 = d0-d2, t1 = d1+d2, t2 = d2-d1, t3 = d1-d3
  m_p = sum_cin sum_kh w'_p[kh] * t_p[row r+kh]
        (w'_0=g0, w'_1=(g0+g1+g2)/2, w'_2=(g0-g1+g2)/2, w'_3=g2)
  out[2u]   = m0+m1+m2+bias
  out[2u+1] = m1-m2-m3+bias
PE work drops from 9 to 6 matmul-columns per output pixel (and the
junk 57-stride column of a direct kernel disappears): 150.5K cols/core
= 62.7us at 2.4GHz vs 95.8us direct.

Layout: the host splits padded rows into even/odd column planes stored
row-major ([58 rows][2 planes][29 cols]) so (a) the four t-plane
transforms are contiguous packed-bf16 tensor_tensor ops on DVE (2x
mode) and (b) row-chunk DMAs write contiguous ranges (the tile dep
tracker uses bounding intervals; an interleaved layout creates false
chunk->transform deps). The t-planes [cin, 58*28] use flat row-stride
28: vertical tap kh of a group at flat col lo is the constant offset
lo + kh*28, so 392-col matmuls run seamlessly across row boundaries.

Per group of N=392 pair-cols: 12 matmuls (4 m-banks x 3 kh) accumulate
in 4 PSUM banks; 2 groups double-buffer across the 8 banks. GpSimd
cannot read PSUM and two-tensor DVE ops may read at most one PSUM
operand, so the A^T combine is:
  ACT:    a1 = Ident(m1+bias), a2 = Copy(m2), a3 = Copy(-m3)
  DVE:    w1 = a1-a2 (sbuf 2x), u0 = m0+a1 (1 psum), out0 = u0+a2
  GpSimd: out1 = w1+a3 (sbuf only)
Outputs stay as separate even/odd bf16 planes, one DMA per plane per
group (per-plane DRAM ranges are contiguous -> no false WAW chain
between group output DMAs); the host interleaves and widens to f32
(tolerance is 2e-2; bf16 out lands ~7e-3).

Startup: warmup matmuls on an uninitialized (dep-free) tile ramp the
PE clock from the first instruction; the first DMA wave (chunk 0 on
SP; half-0 weights + chunk 1 + bias on ACT) lands while they run.
Transfers not needed before ~10us (chunks 2-4, half-1 weights, image
prefetches) are gated behind warmup/first-drain WAW touches so the
Tile scheduler cannot hoist them into the critical wave. Transforms
for image b+1 run on DVE as half-planes spread over 6 group slots of
image b; the split image prefetch feeds them. The final half tapers
(392,392,392,280,112) and its last drain chain avoids the (possibly
backlogged) GpSimd queue and ships its two planes on separate queues.
"""

import numpy as np
import ml_dtypes

import concourse.bacc as bacc
import concourse.mybir as mybir
import concourse.tile as tile
from concourse.bass_utils import run_bass_kernel_spmd
from concourse.alu_op_type import AluOpType

B, CIN, H, W = 32, 128, 56, 56
COUT = 256
NCORES = 8
BLOC = B // NCORES  # images per core
NR = H + 2  # 58 padded rows
PW = W // 2 + 1  # 29 even/odd plane cols
PC = W // 2  # 28 output pairs per row
NPAIR = H * PC  # 1568 output pair-cols per image-half
NWARM = 4

# Weight stationary order per half = first-use order: m1, m2, m0, m3.
WORDER = [1, 2, 0, 3]
PIDX = {p: i for i, p in enumerate(WORDER)}

NORM_GROUPS = [(0, 392), (392, 392), (784, 392), (1176, 392)]
# Image 0 half 0: start-taper so the first matmuls need only 7 input
# rows (the first DMA chunk).
FIRST_GROUPS = [(0, 112), (112, 280), (392, 392), (784, 392), (1176, 392)]
# Last half: end-taper so the final drain + output DMA chain after the
# last matmul is as short as possible.
LAST_GROUPS = [(0, 392), (392, 392), (784, 392), (1176, 280), (1456, 112)]
# Image 0 xe/xo row chunks (DMA + transform granularity).
CHUNKS0 = [(0, 7), (7, 17), (17, 31), (31, 45), (45, 58)]

# Transform schedule for image b+1 during image b, as
# (h, gi) -> [(plane, row_lo, row_hi), ...]. Plane 1 first (used first
# by the next image's group 0), planes 0/3 last (their first use is
# deepest into the next image's group 0).
TSCHED = {
    (0, 2): [(1, 0, 29)],
    (0, 3): [(1, 29, 58)],
    (1, 0): [(2, 0, 29)],
    (1, 1): [(2, 29, 58)],
    (1, 2): [(0, 0, 29), (0, 29, 58)],
    (1, 3): [(3, 0, 29), (3, 29, 58)],
}

_nc_cache = None


def _build():
    f32 = mybir.dt.float32
    bf16 = mybir.dt.bfloat16
    COPY = mybir.ActivationFunctionType.Copy
    IDENT = mybir.ActivationFunctionType.Identity
    nc = bacc.Bacc("TRN2", target_bir_lowering=False)
    x_d = nc.dram_tensor("xeo", [BLOC, CIN, NR, 2, PW], bf16, kind="ExternalInput")
    w_d = nc.dram_tensor("wT", [CIN, 2 * 4 * 3 * 128], bf16, kind="ExternalInput")
    b_d = nc.dram_tensor("bias2", [128, 2], f32, kind="ExternalInput")
    o_d = nc.dram_tensor("out", [BLOC, 2, 128, 2, NPAIR], bf16, kind="ExternalOutput")

    def wcol(h, p, kh):
        return ((h * 4 + PIDX[p]) * 3 + kh) * 128

    with tile.TileContext(nc) as tc:
        with (
            tc.tile_pool(name="wpool", bufs=1) as wpool,
            tc.tile_pool(name="xpool", bufs=2) as xpool,
            tc.tile_pool(name="tpool", bufs=8) as tpool,
            tc.tile_pool(name="upool", bufs=3) as upool,
            tc.tile_pool(name="vpool", bufs=3) as vpool,
            tc.tile_pool(name="opool", bufs=4) as opool,
            tc.tile_pool(name="psum", bufs=8, space="PSUM") as psum,
        ):
            wsb = wpool.tile([CIN, 2 * 4 * 3 * 128], bf16)
            bsb = wpool.tile([128, 2], f32)
            wub = wpool.tile([128, 512], bf16)
            dmy = wpool.tile([128, 2], bf16)
            nc.vector.memset(wub[:], 0.0)
            # Dummy Identity activation: pulls the ~1.3us activation
            # table load to the front of the ACT queue (its engine
            # queue depth is 0, so a late table load would stall it).
            nc.scalar.activation(dmy[:], wub[:, :2], IDENT)

            xeos = [xpool.tile([CIN, NR, 2, PW], bf16, tag="xeo", name="xeo0")]
            tpls = [
                [
                    tpool.tile([CIN, NR, PC], bf16, tag="tp", name=f"tp0_{p}")
                    for p in range(4)
                ]
            ]

            # PE warmup on the uninitialized tile: matmul 1 issues at
            # the first possible cycle and its completion ungates the
            # non-critical DMAs below; 2-4 keep the clock ramping while
            # the first chunks land and transform.
            wup = psum.tile([128, 512], f32, tag="pt", name="wup")
            nc.tensor.matmul(
                wup[:], lhsT=wub[:, :128], rhs=wub[:], start=True, stop=True
            )
            # WAW touches: gate chunks 2-4 + half-1 weights behind
            # warmup matmul 1 (the Tile scheduler hoists dep-free DMAs
            # past queue order, so position alone cannot hold them out
            # of the critical first wave).
            nc.vector.tensor_scalar_mul(wsb[:, 1536:1538], wup[:, :2], 0.0)
            xeo0 = xeos[0]
            for (r0, r1) in CHUNKS0[2:]:
                nc.vector.tensor_scalar_mul(
                    xeo0[:, r0, 0, 0:2], wup[:, :2], 0.0
                )
            for _ in range(NWARM - 1):
                nc.tensor.matmul(
                    wup[:], lhsT=wub[:, :128], rhs=wub[:], start=True, stop=True
                )

            # Startup DMA wave. SP: chunk 0, then gated chunks 2/4.
            # ACT: half-0 weights, chunk 1, bias, chunk 3, gated half-1
            # weights. Two queues generate descriptors in parallel.
            c0, c1 = CHUNKS0[0], CHUNKS0[1]
            c2, c3, c4 = CHUNKS0[2], CHUNKS0[3], CHUNKS0[4]
            nc.sync.dma_start(
                xeo0[:, c0[0] : c0[1]], x_d[0, :, c0[0] : c0[1]]
            )
            nc.scalar.dma_start(wsb[:, 0:1536], w_d[:, 0:1536])
            nc.sync.dma_start(xeo0[:, c2[0] : c2[1]], x_d[0, :, c2[0] : c2[1]])
            nc.scalar.dma_start(
                xeo0[:, c1[0] : c1[1]], x_d[0, :, c1[0] : c1[1]]
            )
            nc.sync.dma_start(xeo0[:, c4[0] : c4[1]], x_d[0, :, c4[0] : c4[1]])
            nc.scalar.dma_start(bsb[:], b_d[:])
            nc.scalar.dma_start(
                xeo0[:, c3[0] : c3[1]], x_d[0, :, c3[0] : c3[1]]
            )
            nc.scalar.dma_start(wsb[:, 1536:], w_d[:, 1536:])

            def transform(bi, r0, r1, only=None):
                """t-plane rows [r0,r1) for image slot bi (DVE)."""
                xeo = xeos[bi]
                tp = tpls[bi]
                xe = lambda a, b_: xeo[:, r0:r1, 0, a:b_]
                xo = lambda a, b_: xeo[:, r0:r1, 1, a:b_]
                ops = {
                    0: (nc.vector.tensor_sub, xe(0, PC), xe(1, PC + 1)),
                    1: (nc.vector.tensor_add, xo(0, PC), xe(1, PC + 1)),
                    2: (nc.vector.tensor_sub, xe(1, PC + 1), xo(0, PC)),
                    3: (nc.vector.tensor_sub, xo(0, PC), xo(1, PC + 1)),
                }
                order = [only] if only is not None else WORDER
                for p in order:
                    fn, a, b_ = ops[p]
                    fn(tp[p][:, r0:r1, :], a, b_)

            transform(0, *CHUNKS0[0])
            transform(0, *CHUNKS0[1])

            def do_group(b, h, lo, n, last_group=False, after_drains=None,
                         prefetch=False):
                tp = tpls[b]
                flat = [tp[p][:].rearrange("c r u -> c (r u)") for p in range(4)]
                pts = {}
                for p in WORDER:
                    pts[p] = psum.tile(
                        [128, 392], f32, tag="pt", name=f"pt_b{b}h{h}l{lo}p{p}"
                    )
                    for kh in range(3):
                        c = wcol(h, p, kh)
                        nc.tensor.matmul(
                            pts[p][:, :n],
                            lhsT=wsb[:, c : c + 128],
                            rhs=flat[p][:, lo + kh * PC : lo + kh * PC + n],
                            start=(kh == 0),
                            stop=(kh == 2),
                        )
                a1 = vpool.tile([128, 392], bf16, tag="a1")
                a2 = vpool.tile([128, 392], bf16, tag="a2")
                a3 = vpool.tile([128, 392], bf16, tag="a3")
                u0 = upool.tile([128, 392], bf16, tag="u0")
                w1 = upool.tile([128, 392], bf16, tag="w1")
                ot = opool.tile([128, 2, 392], bf16, tag="ot")
                bvec = bsb[:, h : h + 1]
                nc.scalar.activation(a1[:, :n], pts[1][:, :n], IDENT, bias=bvec)
                nc.scalar.activation(a2[:, :n], pts[2][:, :n], COPY)
                nc.scalar.activation(a3[:, :n], pts[3][:, :n], COPY, scale=-1.0)
                # out0 = (m0 + a1) + a2 ; out1 = (a1 - a2) + a3
                nc.vector.tensor_sub(w1[:, :n], a1[:, :n], a2[:, :n])
                nc.vector.tensor_add(u0[:, :n], pts[0][:, :n], a1[:, :n])
                nc.vector.tensor_add(ot[:, 0, :n], u0[:, :n], a2[:, :n])
                out1_eng = nc.vector if last_group else nc.gpsimd
                out1_eng.tensor_add(ot[:, 1, :n], w1[:, :n], a3[:, :n])
                if last_group:
                    # Ship the two final planes on different queues so
                    # their descriptor generations overlap.
                    nc.sync.dma_start(o_d[b, h, :, 0, lo : lo + n], ot[:, 0, :n])
                    nc.scalar.dma_start(o_d[b, h, :, 1, lo : lo + n], ot[:, 1, :n])
                else:
                    nc.sync.dma_start(o_d[b, h, :, :, lo : lo + n], ot[:, :, :n])
                if prefetch:
                    # Prefetch next image's xe/xo in two halves, each
                    # gated behind this group's first output plane (an
                    # early 861KB prefetch would starve the transfers
                    # gating the PE; the halves let the first
                    # transforms start 1.2us sooner).
                    xeon = xpool.tile(
                        [CIN, NR, 2, PW], bf16, tag="xeo", name=f"xeo{b+1}"
                    )
                    xeos.append(xeon)
                    tpls.append(
                        [
                            tpool.tile(
                                [CIN, NR, PC], bf16, tag="tp", name=f"tp{b+1}_{p}"
                            )
                            for p in range(4)
                        ]
                    )
                    nc.gpsimd.tensor_scalar_mul(
                        xeon[:, 0, 0, 0:2], ot[:, 0, 0:2], 0.0
                    )
                    nc.sync.dma_start(xeon[:], x_d[b + 1])
                if after_drains is not None:
                    after_drains()

            for b in range(BLOC):
                for h in range(2):
                    if b == 0 and h == 0:
                        groups = FIRST_GROUPS
                    elif b == BLOC - 1 and h == 1:
                        groups = LAST_GROUPS
                    else:
                        groups = NORM_GROUPS
                    for gi, (lo, n) in enumerate(groups):
                        after = None
                        if b == 0 and h == 0 and gi < 3:
                            r0, r1 = CHUNKS0[gi + 2]
                            after = lambda r0=r0, r1=r1: transform(0, r0, r1)
                        elif h == 1 and b + 1 < BLOC and gi < 4:
                            # One whole-plane transform for image b+1
                            # per half-1 group slot, in use order.
                            p = WORDER[gi]
                            after = lambda b=b, p=p: transform(
                                b + 1, 0, NR, only=p
                            )
                        do_group(
                            b, h, lo, n,
                            last_group=(
                                b == BLOC - 1 and h == 1 and gi == len(groups) - 1
                            ),
                            after_drains=after,
                            prefetch=(h == 0 and gi == 0 and b + 1 < BLOC),
                        )

    nc.compile()
    return nc


def _get_nc():
    global _nc_cache
    if _nc_cache is None:
        _nc_cache = _build()
    return _nc_cache


def _prep_inputs(x, weights, bias):
    x = np.asarray(x, dtype=np.float32)
    weights = np.asarray(weights, dtype=np.float32)
    bias = np.ascontiguousarray(np.asarray(bias, dtype=np.float32))

    xb = x.astype(ml_dtypes.bfloat16)
    xpad = np.pad(xb, ((0, 0), (0, 0), (1, 1), (1, 1)))  # [B,C,58,58]
    xe = xpad[:, :, :, 0::2]  # [B,C,58,29]
    xo = xpad[:, :, :, 1::2]
    xeo = np.ascontiguousarray(np.stack([xe, xo], axis=3))  # [B,C,58,2,29]

    g = weights.reshape(2, 128, CIN, 3, 3)  # [h, co, cin, kh, kw]
    w0 = g[..., 0]
    w1 = (g[..., 0] + g[..., 1] + g[..., 2]) * 0.5
    w2 = (g[..., 0] - g[..., 1] + g[..., 2]) * 0.5
    w3 = g[..., 2]
    wlist = [w0, w1, w2, w3]
    # stack in WORDER; axes [h, p, co, cin, kh] -> [cin, h, p, kh, co]
    wp = np.stack([wlist[p] for p in WORDER], axis=1)
    wT = np.ascontiguousarray(wp.transpose(3, 0, 1, 4, 2)).reshape(
        CIN, 2 * 4 * 3 * 128
    ).astype(ml_dtypes.bfloat16)
    b2 = np.ascontiguousarray(bias.reshape(2, 128).T)  # b2[p,h] = bias[h*128+p]

    return [
        {
            "xeo": np.ascontiguousarray(xeo[i * BLOC : (i + 1) * BLOC]),
            "wT": wT,
            "bias2": b2,
        }
        for i in range(NCORES)
    ]


def _run(inputs, trace=False):
    in_maps = _prep_inputs(inputs["x"], inputs["weights"], inputs["bias"])
    res = run_bass_kernel_spmd(
        _get_nc(), in_maps, core_ids=list(range(NCORES)), trace=trace
    )
    o = np.concatenate([np.asarray(r["out"]) for r in res.results], axis=0)
    # [B, 2h, 128co, 2pl, 1568] bf16 -> [B, 256, 56, 56] f32
    o = o.astype(np.float32).reshape(B, 2, 128, 2, H, PC)
    o = o.transpose(0, 1, 2, 4, 5, 3).reshape(B, COUT, H, W)
    return np.ascontiguousarray(o), res


def kernel(x, weights, bias):
    out, _ = _run({"x": x, "weights": weights, "bias": bias})
    return out


# revision 21
# speedup vs baseline: 1.2093x; 1.0273x over previous
"""Conv2d 3x3 (pad 1, stride 1) + bias on 8 Trainium2 cores.

Problem: x [32,128,56,56] f32, weights [256,128,3,3] f32, bias [256] f32
         -> out [32,256,56,56] f32.

Strategy
--------
Data-parallel over batch (4 images/core) + 1D Winograd F(2,3) along W.

For each output pair (2u, 2u+1) and each vertical tap kh, the 3-tap
horizontal conv costs 4 multiplies instead of 6: with d0..d3 the 4
padded inputs around the pair,
  t0 = d0-d2, t1 = d1+d2, t2 = d2-d1, t3 = d1-d3
  m_p = sum_cin sum_kh w'_p[kh] * t_p[row r+kh]
        (w'_0=g0, w'_1=(g0+g1+g2)/2, w'_2=(g0-g1+g2)/2, w'_3=g2)
  out[2u]   = m0+m1+m2+bias
  out[2u+1] = m1-m2-m3+bias
PE work drops from 9 to 6 matmul-columns per output pixel (and the
junk 57-stride column of a direct kernel disappears): 150.5K cols/core
= 62.7us at 2.4GHz vs 95.8us direct.

Layout: the host splits padded rows into even/odd column planes stored
row-major ([58 rows][2 planes][29 cols]) so (a) the four t-plane
transforms are contiguous packed-bf16 tensor_tensor ops on DVE (2x
mode) and (b) row-chunk DMAs write contiguous ranges (the tile dep
tracker uses bounding intervals; an interleaved layout creates false
chunk->transform deps). The t-planes [cin, 58*28] use flat row-stride
28: vertical tap kh of a group at flat col lo is the constant offset
lo + kh*28, so matmuls run seamlessly across row boundaries.

Work unit = a 784-pair-col double-group: each m_p accumulates in a
2-bank PSUM tile via 6 matmuls (3 kh x [0:512]+[512:784] bank-aligned
splits); 4 m-tiles = all 8 banks. There is no group-level double
buffering - instead each PSUM bank has exactly ONE drain reader, so
the next double-group's first matmuls only wait on a drain that
completed mid-previous-group. Wide drains halve the per-op overheads
(the 392-col variant left ACT/DVE/GpSimd at 90-110% of the PE window;
this one leaves every engine at <=80%):
  ACT:    a1 = Ident(m1+bias), a2 = Copy(m2), a3 = Copy(-m3)
  DVE:    w1 = a1-a2 (sbuf 2x), u0 = m0+a1 (the one psum op), out0 = u0+a2
  GpSimd: out1 = w1+a3 (sbuf only; GpSimd cannot read PSUM)
The two cout-halves interleave per double-group (dg0.h0, dg0.h1,
dg1.h0, ...), which doubles the compute runway per input row-chunk at
startup. Outputs stay as separate even/odd bf16 planes (the host
interleaves and widens to f32; tolerance is 2e-2, bf16 out lands
~7e-3), one DMA per double-group.

Startup: warmup matmuls ramp the PE clock while the first DMA wave
lands (SP: chunk 0; ACT: bias, weights in first-use order, chunk 1).
Transfers not needed before ~+4us (chunks 2-3, image prefetches) are
gated behind warmup/ACT-drain WAW touches so the Tile scheduler
cannot hoist them into the critical wave. Image b+1's t-planes are
built on DVE, one whole plane per group slot, spread so each lands
just before its first use; plane 3 of image b is built in image b's
own first slot. The final half tapers (784,512,272) and its last
drain chain avoids the GpSimd queue and ships its two output planes
on separate queues.
"""

import numpy as np
import ml_dtypes

import concourse.bacc as bacc
import concourse.mybir as mybir
import concourse.tile as tile
from concourse.bass_utils import run_bass_kernel_spmd

B, CIN, H, W = 32, 128, 56, 56
COUT = 256
NCORES = 8
BLOC = B // NCORES  # images per core
NR = H + 2  # 58 padded rows
PW = W // 2 + 1  # 29 even/odd plane cols
PC = W // 2  # 28 output pairs per row
NPAIR = H * PC  # 1568 output pair-cols per image-half
NWARM = 4

# Weight stationary order per half = first-use order: m1, m2, m0, m3.
WORDER = [1, 2, 0, 3]
PIDX = {p: i for i, p in enumerate(WORDER)}

NORM_GROUPS = [(0, 784), (784, 784)]
# Image 0: start-taper so the first matmuls need only the first DMA
# row-chunk and weight slice.
FIRST_GROUPS = [(0, 272), (272, 512), (784, 512), (1296, 272)]
# Last image: end-taper so the final drain + output DMA chain after
# the last matmul is as short as possible.
LAST_GROUPS = [(0, 784), (784, 512), (1296, 272)]
# Image 0 xe/xo row chunks (DMA + transform granularity). Group dg_i of
# FIRST_GROUPS needs plane rows < CHUNKS0[i][1].
CHUNKS0 = [(0, 12), (12, 30), (30, 49), (49, 58)]

_nc_cache = None


def _build():
    f32 = mybir.dt.float32
    bf16 = mybir.dt.bfloat16
    COPY = mybir.ActivationFunctionType.Copy
    IDENT = mybir.ActivationFunctionType.Identity
    nc = bacc.Bacc("TRN2", target_bir_lowering=False)
    x_d = nc.dram_tensor("xeo", [BLOC, CIN, NR, 2, PW], bf16, kind="ExternalInput")
    w_d = nc.dram_tensor("wT", [CIN, 2 * 4 * 3 * 128], bf16, kind="ExternalInput")
    b_d = nc.dram_tensor("bias2", [128, 2], f32, kind="ExternalInput")
    o_d = nc.dram_tensor("out", [BLOC, 2, 128, 2, NPAIR], bf16, kind="ExternalOutput")

    def wcol(h, p, kh):
        return ((h * 4 + PIDX[p]) * 3 + kh) * 128

    with tile.TileContext(nc) as tc:
        with (
            tc.tile_pool(name="wpool", bufs=1) as wpool,
            tc.tile_pool(name="xpool", bufs=2) as xpool,
            tc.tile_pool(name="tpool", bufs=8) as tpool,
            tc.tile_pool(name="upool", bufs=3) as upool,
            tc.tile_pool(name="vpool", bufs=3) as vpool,
            tc.tile_pool(name="opool", bufs=3) as opool,
            tc.tile_pool(name="psum", bufs=4, space="PSUM") as psum,
        ):
            wsb = wpool.tile([CIN, 2 * 4 * 3 * 128], bf16)
            bsb = wpool.tile([128, 2], f32)
            wub = wpool.tile([128, 512], bf16)
            dmy = wpool.tile([128, 2], bf16)
            nc.vector.memset(wub[:], 0.0)
            # Dummy Identity activation: pulls the ~1.3us activation
            # table load to the front of the ACT queue (its engine
            # queue depth is 0, so a late table load would stall it).
            nc.scalar.activation(dmy[:], wub[:, :2], IDENT)

            xeos = [xpool.tile([CIN, NR, 2, PW], bf16, tag="xeo", name="xeo0")]
            tpls = [
                [
                    tpool.tile([CIN, NR, PC], bf16, tag="tp", name=f"tp0_{p}")
                    for p in range(4)
                ]
            ]

            # PE warmup: matmul 1 issues as soon as the memset lands
            # and its completion ungates the non-critical DMAs below;
            # 2-4 keep the clock ramping while the first chunks land.
            wup = psum.tile([128, 512], f32, tag="pt", name="wup")
            nc.tensor.matmul(
                wup[:], lhsT=wub[:, :128], rhs=wub[:], start=True, stop=True
            )
            # WAW touches: gate chunks 2-3 behind warmup matmul 1 (the
            # Tile scheduler hoists dep-free DMAs past queue order, so
            # position alone cannot keep them out of the first wave).
            xeo0 = xeos[0]
            for (r0, r1) in CHUNKS0[2:]:
                nc.vector.tensor_scalar_mul(
                    xeo0[:, r0, 0, 0:2], wup[:, :2], 0.0
                )
            for _ in range(NWARM - 1):
                nc.tensor.matmul(
                    wup[:], lhsT=wub[:, :128], rhs=wub[:], start=True, stop=True
                )

            # Startup DMA wave, ordered by first-use deadline. SP:
            # chunk 0 + gated chunks 2-3. ACT: bias, h0 weights, h1
            # weights (halves interleave, so h1 is needed by ~+2us),
            # chunk 1.
            c0, c1, c2, c3 = CHUNKS0
            nc.sync.dma_start(xeo0[:, c0[0] : c0[1]], x_d[0, :, c0[0] : c0[1]])
            nc.scalar.dma_start(bsb[:], b_d[:])
            nc.scalar.dma_start(wsb[:, 0:1536], w_d[:, 0:1536])
            nc.scalar.dma_start(wsb[:, 1536:2304], w_d[:, 1536:2304])
            nc.scalar.dma_start(wsb[:, 2304:3072], w_d[:, 2304:3072])
            nc.sync.dma_start(xeo0[:, c2[0] : c2[1]], x_d[0, :, c2[0] : c2[1]])
            nc.scalar.dma_start(xeo0[:, c1[0] : c1[1]], x_d[0, :, c1[0] : c1[1]])
            nc.sync.dma_start(xeo0[:, c3[0] : c3[1]], x_d[0, :, c3[0] : c3[1]])

            def transform(bi, r0, r1, only=None):
                """t-plane rows [r0,r1) for image slot bi (DVE)."""
                xeo = xeos[bi]
                tp = tpls[bi]
                xe = lambda a, b_: xeo[:, r0:r1, 0, a:b_]
                xo = lambda a, b_: xeo[:, r0:r1, 1, a:b_]
                ops = {
                    0: (nc.vector.tensor_sub, xe(0, PC), xe(1, PC + 1)),
                    1: (nc.vector.tensor_add, xo(0, PC), xe(1, PC + 1)),
                    2: (nc.vector.tensor_sub, xe(1, PC + 1), xo(0, PC)),
                    3: (nc.vector.tensor_sub, xo(0, PC), xo(1, PC + 1)),
                }
                order = [only] if only is not None else WORDER
                for p in order:
                    fn, a, b_ = ops[p]
                    fn(tp[p][:, r0:r1, :], a, b_)

            transform(0, *CHUNKS0[0])

            def do_group(b, h, lo, n, last_group=False, pre_drains=None,
                         pre_mm=None, prefetch=False):
                if pre_mm is not None:
                    pre_mm()
                tp = tpls[b]
                flat = [tp[p][:].rearrange("c r u -> c (r u)") for p in range(4)]
                ranges = [(0, min(n, 512))] + ([(512, n)] if n > 512 else [])
                pts = {}
                for p in WORDER:
                    pts[p] = psum.tile(
                        [128, 784], f32, tag="pt", name=f"pt_b{b}h{h}l{lo}p{p}"
                    )
                    for kh in range(3):
                        c = wcol(h, p, kh)
                        for (r0, r1) in ranges:
                            nc.tensor.matmul(
                                pts[p][:, r0:r1],
                                lhsT=wsb[:, c : c + 128],
                                rhs=flat[p][:, lo + kh * PC + r0 : lo + kh * PC + r1],
                                start=(kh == 0),
                                stop=(kh == 2),
                            )
                a1 = vpool.tile([128, 784], bf16, tag="a1")
                a2 = vpool.tile([128, 784], bf16, tag="a2")
                a3 = vpool.tile([128, 784], bf16, tag="a3")
                u0 = upool.tile([128, 784], bf16, tag="u0")
                w1 = upool.tile([128, 784], bf16, tag="w1")
                ot = opool.tile([128, 2, 784], bf16, tag="ot")
                bvec = bsb[:, h : h + 1]
                nc.scalar.activation(a1[:, :n], pts[1][:, :n], IDENT, bias=bvec)
                nc.scalar.activation(a2[:, :n], pts[2][:, :n], COPY)
                nc.scalar.activation(a3[:, :n], pts[3][:, :n], COPY, scale=-1.0)
                if prefetch:
                    # Prefetch next image's xe/xo, gated behind this
                    # group's a1 (an early 861KB prefetch would starve
                    # the transfers gating the PE).
                    xeon = xpool.tile(
                        [CIN, NR, 2, PW], bf16, tag="xeo", name=f"xeo{b+1}"
                    )
                    xeos.append(xeon)
                    tpls.append(
                        [
                            tpool.tile(
                                [CIN, NR, PC], bf16, tag="tp", name=f"tp{b+1}_{p}"
                            )
                            for p in range(4)
                        ]
                    )
                    nc.gpsimd.tensor_scalar_mul(
                        xeon[:, 0, 0, 0:2], a1[:, 0:2], 0.0
                    )
                    nc.sync.dma_start(xeon[:], x_d[b + 1])
                if pre_drains is not None:
                    pre_drains()
                # out0 = (m0 + a1) + a2 ; out1 = (a1 - a2) + a3
                nc.vector.tensor_sub(w1[:, :n], a1[:, :n], a2[:, :n])
                nc.vector.tensor_add(u0[:, :n], pts[0][:, :n], a1[:, :n])
                nc.vector.tensor_add(ot[:, 0, :n], u0[:, :n], a2[:, :n])
                out1_eng = nc.vector if last_group else nc.gpsimd
                out1_eng.tensor_add(ot[:, 1, :n], w1[:, :n], a3[:, :n])
                if last_group:
                    nc.sync.dma_start(o_d[b, h, :, 0, lo : lo + n], ot[:, 0, :n])
                    nc.scalar.dma_start(o_d[b, h, :, 1, lo : lo + n], ot[:, 1, :n])
                else:
                    nc.sync.dma_start(o_d[b, h, :, :, lo : lo + n], ot[:, :, :n])

            # Transform jobs per (image, slot): slot = dg_index*2 + h.
            # Image 0's slots also carry its chunked transforms; plane
            # 3 of image b is built in image b's own slot 0 (its first
            # use is ~3us into the slot); planes 1/2/0 for image b+1
            # land in image b's later slots, after its prefetch.
            def whole(bi, p):
                return lambda bi=bi, p=p: transform(bi, 0, NR, only=p)

            def chunk(ci):
                return lambda ci=ci: transform(0, *CHUNKS0[ci])

            jobs = {b: {} for b in range(BLOC)}
            jobs[0][1] = [chunk(1)]
            jobs[0][3] = [chunk(2)]
            jobs[0][4] = [chunk(3)]
            for b in range(BLOC):
                if b > 0:
                    jobs[b][0] = [whole(b, 3)]
                if b + 1 < BLOC:
                    base = 5 if b == 0 else 1
                    jobs[b][base] = [whole(b + 1, 1)]
                    jobs[b][base + 1] = [whole(b + 1, 2)]
                    jobs[b][base + 2] = [whole(b + 1, 0)]

            for b in range(BLOC):
                if b == 0:
                    groups = FIRST_GROUPS
                elif b == BLOC - 1:
                    groups = LAST_GROUPS
                else:
                    groups = NORM_GROUPS
                for dgi, (lo, n) in enumerate(groups):
                    for h in range(2):
                        slot = dgi * 2 + h
                        jl = jobs[b].get(slot)
                        pre = (
                            None if not jl
                            else (lambda jl=jl: [f() for f in jl])
                        )
                        # Slot-0 jobs build a plane this group's own
                        # matmuls read - emit them BEFORE the matmuls
                        # (dep tracking is program-order based).
                        do_group(
                            b, h, lo, n,
                            last_group=(
                                b == BLOC - 1
                                and dgi == len(groups) - 1
                                and h == 1
                            ),
                            pre_drains=pre if slot != 0 else None,
                            pre_mm=pre if slot == 0 else None,
                            prefetch=(
                                dgi == 0 and h == 0 and b + 1 < BLOC
                            ),
                        )

    nc.compile()
    return nc


def _get_nc():
    global _nc_cache
    if _nc_cache is None:
        _nc_cache = _build()
    return _nc_cache


def _prep_inputs(x, weights, bias):
    x = np.asarray(x, dtype=np.float32)
    weights = np.asarray(weights, dtype=np.float32)
    bias = np.ascontiguousarray(np.asarray(bias, dtype=np.float32))

    xb = x.astype(ml_dtypes.bfloat16)
    xpad = np.pad(xb, ((0, 0), (0, 0), (1, 1), (1, 1)))  # [B,C,58,58]
    xe = xpad[:, :, :, 0::2]  # [B,C,58,29]
    xo = xpad[:, :, :, 1::2]
    xeo = np.ascontiguousarray(np.stack([xe, xo], axis=3))  # [B,C,58,2,29]

    g = weights.reshape(2, 128, CIN, 3, 3)  # [h, co, cin, kh, kw]
    w0 = g[..., 0]
    w1 = (g[..., 0] + g[..., 1] + g[..., 2]) * 0.5
    w2 = (g[..., 0] - g[..., 1] + g[..., 2]) * 0.5
    w3 = g[..., 2]
    wlist = [w0, w1, w2, w3]
    # stack in WORDER; axes [h, p, co, cin, kh] -> [cin, h, p, kh, co]
    wp = np.stack([wlist[p] for p in WORDER], axis=1)
    wT = np.ascontiguousarray(wp.transpose(3, 0, 1, 4, 2)).reshape(
        CIN, 2 * 4 * 3 * 128
    ).astype(ml_dtypes.bfloat16)
    b2 = np.ascontiguousarray(bias.reshape(2, 128).T)  # b2[p,h] = bias[h*128+p]

    return [
        {
            "xeo": np.ascontiguousarray(xeo[i * BLOC : (i + 1) * BLOC]),
            "wT": wT,
            "bias2": b2,
        }
        for i in range(NCORES)
    ]


def _run(inputs, trace=False):
    in_maps = _prep_inputs(inputs["x"], inputs["weights"], inputs["bias"])
    res = run_bass_kernel_spmd(
        _get_nc(), in_maps, core_ids=list(range(NCORES)), trace=trace
    )
    o = np.concatenate([np.asarray(r["out"]) for r in res.results], axis=0)
    # [B, 2h, 128co, 2pl, 1568] bf16 -> [B, 256, 56, 56] f32
    o = o.astype(np.float32).reshape(B, 2, 128, 2, H, PC)
    o = o.transpose(0, 1, 2, 4, 5, 3).reshape(B, COUT, H, W)
    return np.ascontiguousarray(o), res


def kernel(x, weights, bias):
    out, _ = _run({"x": x, "weights": weights, "bias": bias})
    return out
